# revision 7
# baseline (speedup 1.0000x reference)
"""Contrastive-loss kernel for Trainium2, 8 NeuronCores, Bass/Tile.

Math (reference):
    pk, pv: [N, D]; nv: [M, D]   (N = M = 8192, D = 256, fp32)
    pkn = normalize_rows(pk); pvn = normalize_rows(pv); nvn = normalize_rows(nv)
    pos_sim = rowsum(pkn * pvn)                       # [N]
    neg_sim = pkn @ nvn.T                             # [N, M]
    loss = -mean(pos_sim / 2) + mean(logsumexp(neg_sim / 2, axis=-1))
         = (1/N) * sum_i [ lse_i - pos_sim_i / 2 ]

Sharding: rows of pk/pv are split across 8 cores (1024 rows each);
nv is replicated.  Each core returns the scalar partial
sum_i_local(lse_i - pos_sim_i/2); the host sums partials and divides by N.

Per-core algorithm:
  - pk is NOT normalized before the matmul; its row scale s_i = 1/||pk_i||
    is folded into the exp via ACT's per-partition `scale` operand
    (exp(s_i/2 * g_ij)).  rsqrt is computed as exp(-0.5 * ln(x)) so the
    whole kernel uses only the Exp/Ln ACT table set (no table thrash).
  - nv IS normalized (column scales can't be folded into ACT), converted to
    bf16, stored to DRAM scratch in two 128-column halves, and loaded back
    transposed with the xbar DMA transpose to get nvT[k] in [d, j] layout
    (contraction dim on partitions).  Same trick for raw-bf16 pk.
  - G[128, 2048] tiles accumulate in PSUM over the two 128-deep K halves;
    ACT does exp in place on PSUM with accum_out producing the per-row
    partial sums; a final Ln + reductions + a ones-matmul collapse
    everything to a [1,1] scalar.
"""

import numpy as np

import concourse.bacc as bacc
import concourse.bass as bass
import concourse.mybir as mybir
import concourse.tile as tile

N, M, D = 8192, 8192, 256
NCORES = 8
NSHARD = N // NCORES  # 1024
P = 128
T = NSHARD // P  # 8 rows per partition in the chunked [P, T, D] layout
CHUNK = P * T  # 1024 rows per load chunk
N_NV_CHUNKS = M // CHUNK  # 8
RB = NSHARD // P  # 8 row-blocks of the pk shard (== T)
CC = 2048  # G-tile column chunk (4 PSUM banks)
N_CC = M // CC  # 4
NV_CHUNKS_PER_CC = CC // CHUNK  # 2

FP32 = mybir.dt.float32
BF16 = mybir.dt.bfloat16
ADD = mybir.AluOpType.add
MULT = mybir.AluOpType.mult
AF = mybir.ActivationFunctionType
AX = mybir.AxisListType


def _rsqrt(nc, pool, src, tag):
    """rsqrt(src) for a small [P, w] fp32 tile via exp(-0.5*ln(x)).

    Uses only the Ln/Exp ACT functions so the whole kernel stays on one
    ACT table set."""
    w = src.shape[1]
    lnv = pool.tile([P, w], FP32, tag=f"{tag}_ln")
    out = pool.tile([P, w], FP32, tag=f"{tag}_rs")
    nc.scalar.activation(out=lnv, in_=src, func=AF.Ln)
    nc.scalar.activation(out=out, in_=lnv, func=AF.Exp, scale=-0.5)
    return out


def build_nc():
    # Bacc (not plain Bass): its finalize() runs the wait-splitting passes
    # (move_matmul_waits_to_ldweights, generate_event_semaphores) that the
    # TRN2 codegen requires — at most one sync wait per instruction.
    nc = bacc.Bacc("TRN2", target_bir_lowering=False, debug=False)
    pk = nc.declare_dram_parameter("pk", [NSHARD, D], FP32, isOutput=False)
    pv = nc.declare_dram_parameter("pv", [NSHARD, D], FP32, isOutput=False)
    nv = nc.declare_dram_parameter("nv", [M, D], FP32, isOutput=False)
    out = nc.declare_dram_parameter("partial", [1, 1], FP32, isOutput=True)

    # bf16 DRAM scratch, split into the two 128-wide d-halves so the xbar
    # transpose reads each half fully contiguously.
    nvn_half = [nc.dram_tensor(f"nvn{k}", [M, P], BF16) for k in range(2)]
    pkb_half = [nc.dram_tensor(f"pkb{k}", [NSHARD, P], BF16) for k in range(2)]

    with (
        tile.TileContext(nc) as tc,
        tc.tile_pool(name="big", bufs=3) as big,
        tc.tile_pool(name="half", bufs=2) as half,
        tc.tile_pool(name="nvt", bufs=1) as nvt_pool,
        tc.tile_pool(name="small", bufs=4) as small,
        tc.tile_pool(name="acc", bufs=1) as acc,
        tc.tile_pool(name="psum", bufs=2, space="PSUM") as psum,
    ):
        # ---------------- pk / pv phase ----------------
        # Chunked layout: tile[p, t, :] = shard row 8p + t.
        xk = big.tile([P, T, D], FP32, tag="pk")
        xv = big.tile([P, T, D], FP32, tag="pv")
        nc.sync.dma_start(out=xk, in_=pk.rearrange("(p t) d -> p t d", p=P))
        nc.sync.dma_start(out=xv, in_=pv.rearrange("(p t) d -> p t d", p=P))

        ssk = small.tile([P, T], FP32, tag="ssk")
        ssv = small.tile([P, T], FP32, tag="ssv")
        dot = small.tile([P, T], FP32, tag="dot")
        # scalar_tensor_tensor (standard TENSOR_SCALAR_PTR ISA, not a custom
        # DVE op — those crash this runtime): out = (in0*1.0)*in1, accum=sum
        for t in range(T):
            for in0, in1, accum in ((xk, xk, ssk), (xv, xv, ssv), (xk, xv, dot)):
                sq = big.tile([P, D], FP32, tag="sq")
                nc.vector.scalar_tensor_tensor(
                    out=sq, in0=in0[:, t, :], scalar=1.0, in1=in1[:, t, :],
                    op0=MULT, op1=MULT, accum_out=accum[:, t : t + 1],
                )

        # pos_sim = dot * rsqrt(ssk*ssv); pos_acc[p] = sum_t pos_sim[p, t]
        sskv = small.tile([P, T], FP32, tag="sskv")
        nc.vector.tensor_mul(out=sskv, in0=ssk, in1=ssv)
        sr = _rsqrt(nc, small, sskv, "sr")
        pos = small.tile([P, T], FP32, tag="pos")
        nc.vector.tensor_mul(out=pos, in0=dot, in1=sr)
        pos_acc = acc.tile([P, 1], FP32, tag="pos_acc")
        nc.vector.tensor_reduce(out=pos_acc, in_=pos, axis=AX.X, op=ADD)

        # s_half[p, t] = 0.5 / ||pk row 8p+t||  (the ACT exp scale)
        s_pk = _rsqrt(nc, small, ssk, "spk")
        s_half = acc.tile([P, T], FP32, tag="s_half")
        nc.vector.tensor_scalar_mul(out=s_half, in0=s_pk, scalar1=0.5)

        # raw pk -> bf16, stored interleaved so that pkb row (t*128 + p) holds
        # shard row (8p + t); then matmul row-block rb=t pairs output
        # partition q with s_half[q, rb].
        for k in range(2):
            ykb = half.tile([P, T, P], BF16, tag=f"pkb{k}")
            nc.vector.tensor_copy(out=ykb, in_=xk[:, :, k * P : (k + 1) * P])
            nc.sync.dma_start(
                out=pkb_half[k].rearrange("(t p) d -> p t d", p=P), in_=ykb
            )

        pkT = []
        for k in range(2):
            pt = nvt_pool.tile([P, NSHARD], BF16, tag=f"pkT{k}")
            nc.sync.dma_start_transpose(out=pt, in_=pkb_half[k][:, :])
            pkT.append(pt)

        # ---------------- nv normalize phase + main loop ----------------
        se_all = acc.tile([P, RB, N_CC], FP32, tag="se_all")

        nvT = [[None, None] for _ in range(N_CC)]
        for c in range(N_CC):
            for a_local in range(NV_CHUNKS_PER_CC):
                a = c * NV_CHUNKS_PER_CC + a_local
                rows = slice(a * CHUNK, (a + 1) * CHUNK)
                x = big.tile([P, T, D], FP32, tag="nv")
                nc.sync.dma_start(
                    out=x, in_=nv[rows, :].rearrange("(p t) d -> p t d", p=P)
                )
                ss = small.tile([P, T], FP32, tag="nv_ss")
                for t in range(T):
                    sq = big.tile([P, D], FP32, tag="sq")
                    nc.vector.scalar_tensor_tensor(
                        out=sq, in0=x[:, t, :], scalar=1.0, in1=x[:, t, :],
                        op0=MULT, op1=MULT, accum_out=ss[:, t : t + 1],
                    )
                s = _rsqrt(nc, small, ss, "nv_s")
                for k in range(2):
                    y = half.tile([P, T, P], BF16, tag=f"nvn{k}")
                    for t in range(T):
                        nc.vector.tensor_scalar_mul(
                            out=y[:, t, :],
                            in0=x[:, t, k * P : (k + 1) * P],
                            scalar1=s[:, t : t + 1],
                        )
                    # contiguous store: rows 8p..8p+8 of this chunk live in
                    # partition p, so DRAM side is one 2KB run per partition
                    nc.sync.dma_start(
                        out=nvn_half[k][rows, :].rearrange(
                            "(p t) d -> p t d", p=P
                        ),
                        in_=y,
                    )
            # transposed load of this column chunk: nvT[c][k][d, j]
            for k in range(2):
                nt = nvt_pool.tile([P, CC], BF16, tag=f"nvT{c}_{k}")
                nc.sync.dma_start_transpose(
                    out=nt, in_=nvn_half[k][c * CC : (c + 1) * CC, :]
                )
                nvT[c][k] = nt

            # main loop for this column chunk: all row blocks
            for rb in range(RB):
                g = psum.tile([P, CC], FP32, tag="g")
                for k in range(2):
                    lhsT = pkT[k][:, rb * P : (rb + 1) * P]
                    for n in range(CC // 512):
                        nc.tensor.matmul(
                            g[:, n * 512 : (n + 1) * 512],
                            lhsT,
                            nvT[c][k][:, n * 512 : (n + 1) * 512],
                            start=(k == 0),
                            stop=(k == 1),
                        )
                # exp in place on PSUM; accum_out -> per-row partial sums
                nc.scalar.activation(
                    out=g, in_=g, func=AF.Exp,
                    scale=s_half[:, rb : rb + 1],
                    accum_out=se_all[:, rb, c : c + 1],
                )

        # ---------------- final reduction ----------------
        sume = small.tile([P, RB], FP32, tag="sume")
        nc.vector.tensor_reduce(out=sume, in_=se_all, axis=AX.X, op=ADD)
        lse8 = small.tile([P, RB], FP32, tag="lse8")
        nc.scalar.activation(out=lse8, in_=sume, func=AF.Ln)
        lse_sum = small.tile([P, 1], FP32, tag="lse_sum")
        nc.vector.tensor_reduce(out=lse_sum, in_=lse8, axis=AX.X, op=ADD)

        # total[p] = lse_sum[p] - 0.5 * pos_acc[p]
        neg_half_pos = small.tile([P, 1], FP32, tag="nhp")
        nc.vector.tensor_scalar_mul(out=neg_half_pos, in0=pos_acc, scalar1=-0.5)
        total = small.tile([P, 1], FP32, tag="total")
        nc.vector.tensor_add(out=total, in0=lse_sum, in1=neg_half_pos)

        # cross-partition sum via ones-matmul: [1,1] = total.T @ ones
        ones = small.tile([P, 1], FP32, tag="ones")
        nc.vector.memset(ones, 1.0)
        g_fin = psum.tile([P, CC], FP32, tag="g")
        nc.tensor.matmul(g_fin[0:1, 0:1], total, ones, start=True, stop=True)
        res = small.tile([1, 1], FP32, tag="res")
        nc.vector.tensor_copy(out=res, in_=g_fin[0:1, 0:1])
        nc.sync.dma_start(out=out[:, :], in_=res)

    nc.finalize()
    return nc


_NC_CACHE = None


def _get_nc():
    global _NC_CACHE
    if _NC_CACHE is None:
        _NC_CACHE = build_nc()
    return _NC_CACHE


def make_in_maps(pos_key, pos_value, neg_value):
    pos_key = np.ascontiguousarray(np.asarray(pos_key, dtype=np.float32))
    pos_value = np.ascontiguousarray(np.asarray(pos_value, dtype=np.float32))
    neg_value = np.ascontiguousarray(np.asarray(neg_value, dtype=np.float32))
    return [
        {
            "pk": pos_key[c * NSHARD : (c + 1) * NSHARD],
            "pv": pos_value[c * NSHARD : (c + 1) * NSHARD],
            "nv": neg_value,
        }
        for c in range(NCORES)
    ]


def run(pos_key, pos_value, neg_value, trace=False):
    from concourse.bass_utils import run_bass_kernel_spmd

    nc = _get_nc()
    in_maps = make_in_maps(pos_key, pos_value, neg_value)
    res = run_bass_kernel_spmd(
        nc, in_maps, core_ids=list(range(NCORES)), trace=trace
    )
    partials = np.array(
        [res.results[c]["partial"][0, 0] for c in range(NCORES)],
        dtype=np.float64,
    )
    loss = np.float32(partials.sum() / N)
    return np.asarray(loss, dtype=np.float32), res


def kernel(pos_key, pos_value, neg_value):
    loss, _ = run(pos_key, pos_value, neg_value, trace=False)
    return loss
